# revision 4
# baseline (speedup 1.0000x reference)
"""YOLOv2-style loss (nn_CostYoloV2) on 8 Trainium2 NeuronCores — v2.

Data-parallel over batch (8 batches/core); the noobj term (batch 63 only)
is sharded over the 1024 spatial cells (128 cells/core).

Layout strategy: the truth-cell gather indices are pure host data
(computed from `truth` alone), so the host packs the valid (batch, truth)
rows per core into a [128 part, BLK block, 128 ch] tile and the device
consumes them directly — measured on-device alternatives (streaming all
23 used channels of x + gpsimd indirect_copy + PE transpose: ~26us of DMA
at the 2-queue HWDGE ceiling; swdge dma_gather: ~15us of descriptor
generation; trn2's Pool engine has no elementwise ALU) are all slower.
Rows with valid=0 and all-zero truth slots are pruned host-side (their
loss contributions are provably zero), shrinking both DMA and DVE work.

Per-(b,t) row layout (128 f32): [w x5 anchors | h x5 | obj0 | pad x5 |
cls x20 anchor-innermost | pad x12].  Per-(b,t) scalars ride in a side
tile.  Engine split: DVE runs both the per-(b,t) iou/argmax/coord/class
chain and the noobj mask chain, ACT does relus/squares/accumulations,
PE broadcasts the truth-corner constants across partitions.  Scalar
partials return per core and are combined on the host (the "all-reduce"
of the loss terms).
"""
import numpy as np
import ml_dtypes
from contextlib import ExitStack

import concourse.bass as bass
import concourse.bacc as bacc
import concourse.mybir as mybir
import concourse.tile as tile
from concourse.bass_utils import run_bass_kernel_spmd

F32 = mybir.dt.float32
BF16 = mybir.dt.bfloat16
Alu = mybir.AluOpType
Ax = mybir.AxisListType
Act = mybir.ActivationFunctionType

B, NUM, CLASSES, AL = 64, 5, 20, 25
H = W = 32
HW = H * W
T = 50
NCORES = 8
BL = B // NCORES       # local batches per core
ELEM = 128             # row width (channels, padded)
CELLS = HW // NCORES   # 128 noobj cells per core

_CACHED = {}


def _build_program(blk, tn):
    """blk: number of 128-row blocks of packed (b,t) rows; tn: live truths
    of batch 63 for the noobj term."""
    nc = bacc.Bacc(enable_partition_id=False)
    tvw = 7 * blk
    o_pp = 0                           # pf32 = p63 | tv | xgf
    o_tv = 21
    o_xg = o_tv + tvw
    nf = o_xg + blk * 16
    nb = 5 * tn + blk * CLASSES * NUM  # pbf = tbs | cls
    pf32 = nc.declare_dram_parameter("pf32", [128, nf], F32, isOutput=False)
    pbf = nc.declare_dram_parameter("pbf", [128, nb], BF16, isOutput=False)
    outd = nc.declare_dram_parameter("out", [128, 16], F32, isOutput=True)

    o_wtht, o_at = 0, 2 * blk
    o_twth, o_s2v, o_txys = 3 * blk, 5 * blk, 6 * blk

    with tile.TileContext(nc) as tc, ExitStack() as ctx:
        pool = ctx.enter_context(tc.tile_pool(name="p", bufs=1))

        # -------- input DMAs ------------------------------------------
        itf = pool.tile([128, nf], F32)
        itb = pool.tile([128, nb], BF16)
        nc.sync.dma_start(itb[:, 0:5 * tn], pbf[:, 0:5 * tn])  # tbs
        nc.scalar.dma_start(itf[:, 0:o_xg], pf32[:, 0:o_xg])   # p63|tv
        nc.scalar.dma_start(itf[:, o_xg:], pf32[:, o_xg:])     # xgf
        nc.scalar.dma_start(itb[:, 5 * tn:], pbf[:, 5 * tn:])  # cls
        xt = itf[:, o_xg:o_xg + blk * 16]
        tbs = itb[:, 0:5 * tn]
        clsr = itb[:, 5 * tn:]
        pp = itf[:, o_pp:o_pp + 21]
        tv = itf[:, o_tv:o_tv + tvw]

        out = pool.tile([128, 16], F32)
        nc.gpsimd.memset(out[:], 0.0)

        def tb_b(col):  # truth block [128,tn] -> [128,5,tn] (bcast anchors)
            return tbs[:, col * tn:(col + 1) * tn].rearrange(
                "p (o f) -> p o f", o=1).broadcast_to([128, NUM, tn])

        # ---------------- noobj (batch 63, this core's cells) -------------
        pv = pp[:, 0:20].rearrange("p (n c) -> p n c", c=4)
        xc, yc, wc, hc = pv[:, :, 0], pv[:, :, 1], pv[:, :, 2], pv[:, :, 3]
        corn = pool.tile([128, 25], F32)
        lo = corn[:, 0:10].rearrange("p (g n) -> p n g", g=2)    # al|au
        hi = corn[:, 10:20].rearrange("p (g n) -> p n g", g=2)   # ar|ad
        hap3 = corn[:, 20:25]
        pxy = pv[:, :, 0:2]
        pwh = pv[:, :, 2:4]
        nc.vector.scalar_tensor_tensor(lo, pwh, -0.5, pxy, Alu.mult, Alu.add)
        nc.vector.scalar_tensor_tensor(hi, pwh, 0.5, pxy, Alu.mult, Alu.add)
        # hap/1.5 = wc*hc/3  (mask test: iou>0.5 <=> inter-hat/1.5 > hap/1.5)
        nc.vector.scalar_tensor_tensor(hap3, wc, 1.0 / 3.0, hc,
                                       Alu.mult, Alu.mult)

        al, au = corn[:, 0:5], corn[:, 5:10]
        ar, ad = corn[:, 10:15], corn[:, 15:20]

        def cn_b(apv):  # corner col [128,5] -> [128,5,tn]
            return apv.broadcast_to([128, NUM, tn])

        u1 = pool.tile([128, NUM * tn], F32)
        u2 = pool.tile([128, NUM * tn], F32)
        iw = pool.tile([128, NUM * tn], F32)
        ih = pool.tile([128, NUM * tn], F32)

        def w3(tl):
            return tl[:].rearrange("p (n t) -> p n t", t=tn)

        nc.vector.tensor_tensor(w3(u1), tb_b(2), cn_b(ar), Alu.min)
        nc.vector.tensor_tensor(w3(u2), tb_b(0), cn_b(al), Alu.max)
        nc.vector.tensor_tensor(iw[:], u1[:], u2[:], Alu.subtract)
        nc.vector.tensor_tensor(w3(u1), tb_b(3), cn_b(ad), Alu.min)
        nc.vector.tensor_tensor(w3(u2), tb_b(1), cn_b(au), Alu.max)
        nc.vector.tensor_tensor(ih[:], u1[:], u2[:], Alu.subtract)
        nc.scalar.activation(ih[:], ih[:], Act.Relu)
        nc.vector.tensor_tensor(iw[:], iw[:], ih[:], Alu.mult)        # inter
        # g = inter - hat/1.5 ; mask_n = (max_t g) > hap/1.5
        nc.vector.tensor_tensor(w3(iw), w3(iw), tb_b(4), Alu.subtract)
        anyt = pool.tile([128, 6], F32)
        nc.vector.tensor_reduce(anyt[:, 0:5], w3(iw), Ax.X, Alu.max)
        nc.vector.tensor_tensor(anyt[:, 0:5], anyt[:, 0:5], hap3, Alu.is_gt)
        nc.vector.tensor_reduce(anyt[:, 5:6], anyt[:, 0:5], Ax.X, Alu.add,
                                negate=True)                      # -cnt
        oq2 = pool.tile([128, 2], F32)
        nc.scalar.square(oq2[:, 0:1], pp[:, 20:21])
        nc.scalar.activation(oq2[:, 1:2], pp[:, 20:21], Act.Square,
                             scale=float(NUM) ** 0.5)             # 5*obj^2
        nc.vector.scalar_tensor_tensor(out[:, 12:13], anyt[:, 5:6],
                                       oq2[:, 0:1], oq2[:, 1:2],
                                       Alu.mult, Alu.add)

        # ---------------- warm coords (batch 63 cells) --------------------
        scr = pool.tile([128, 20], F32)
        nc.scalar.activation(scr[:], pp[:, 0:20], Act.Square,
                             accum_out=out[:, 13:14])
        xysc = pool.tile([128, 10], F32)
        nc.scalar.activation(
            xysc[:].rearrange("p (n g) -> p n g", g=2), pv[:, :, 0:2],
            Act.Copy, accum_out=out[:, 14:15])

        # ---------------- per-(b,t) losses — DVE --------------------------
        x3 = xt.rearrange("p (c e) -> p c e", e=16)
        wv = x3[:, :, 0:5]
        hv = x3[:, :, 5:10]
        objv = x3[:, :, 10]                          # [128,blk]
        wh4 = x3[:, :, 0:10].rearrange("p c (g n) -> p c g n", g=2)
        vctv = x3[:, :, 11:16]                       # [128,blk,5]

        wtht_b = tv[:, o_wtht:o_wtht + 2 * blk].rearrange(
            "p (c g o) -> p c g o", g=2, o=1).broadcast_to([128, blk, 2, NUM])
        at_b = tv[:, o_at:o_at + blk].broadcast_to([128, blk, NUM])
        twth = tv[:, o_twth:o_twth + 2 * blk]
        s2v = tv[:, o_s2v:o_s2v + blk]
        txys = tv[:, o_txys:o_txys + blk]               # txy2*s2 (host-folded)

        t1 = pool.tile([128, blk * NUM], F32)
        t2 = pool.tile([128, blk * 2 * NUM], F32)
        t3 = pool.tile([128, blk * NUM], F32)
        t4 = pool.tile([128, blk * NUM], F32)
        mx = pool.tile([128, blk], F32)
        wbhb = pool.tile([128, blk * 2], F32)
        cdf = pool.tile([128, blk], F32)
        t6 = pool.tile([128, blk * 2 * NUM], F32)
        t1b = pool.tile([128, blk * NUM], BF16)
        sqv = pool.tile([128, blk * CLASSES * NUM], F32)
        sst = pool.tile([128, blk * NUM], F32)

        def b5(tl):
            return tl[:].rearrange("p (c n) -> p c n", n=NUM)

        def b20(tl):
            return tl[:].rearrange("p (c e) -> p c e", e=CLASSES)

        t12 = t2[:].rearrange("p (c g n) -> p c g n", g=2, n=NUM)
        nc.vector.tensor_tensor(t12, wh4, wtht_b, Alu.min)
        nc.vector.tensor_tensor(b5(t3), t12[:, :, 0], t12[:, :, 1],
                                Alu.mult)                          # inter
        # argmax_n inter/union == argmax_n inter/(A_n+B): x/(S-x) is
        # monotone in x/S, so the -inter term of the union drops out.
        nc.vector.tensor_tensor(b5(t4), wv, hv, Alu.mult)
        nc.vector.tensor_tensor(b5(t4), b5(t4), at_b, Alu.add)     # A+B
        nc.vector.reciprocal(t4[:], t4[:])
        nc.vector.tensor_tensor(t3[:], t3[:], t4[:], Alu.mult)     # score
        nc.vector.tensor_reduce(mx[:], b5(t3), Ax.X, Alu.max)
        nc.vector.tensor_tensor(b5(t1b), b5(t3),
                                mx[:].broadcast_to([128, blk, NUM]), Alu.is_ge)
        m4 = t1b[:].rearrange("p (c o n) -> p c o n", o=1, n=NUM)
        t6wh = t6[:, 0:blk * 2 * NUM].rearrange(
            "p (c g n) -> p c g n", g=2, n=NUM)
        nc.vector.tensor_tensor(
            t6wh, wh4, m4.broadcast_to([128, blk, 2, NUM]), Alu.mult)
        nc.vector.tensor_reduce(
            wbhb[:].rearrange("p (c g) -> p c g", g=2), t6wh, Ax.X, Alu.add)
        # coords: ((tw-wb)^2 + (th-hb)^2)*s2 + txy2*s2
        nc.vector.tensor_tensor(wbhb[:], twth, wbhb[:], Alu.subtract)
        nc.vector.tensor_tensor(wbhb[:], wbhb[:], wbhb[:], Alu.mult)
        nc.vector.tensor_reduce(cdf[:], wbhb[:].rearrange(
            "p (c g) -> p c g", g=2), Ax.X, Alu.add)
        nc.vector.tensor_tensor(cdf[:], cdf[:], s2v, Alu.mult)
        nc.vector.tensor_tensor(out[:, 0:blk], cdf[:], txys, Alu.add)
        # obj: (1-obj)^2; pad rows contribute exactly 1 (host-subtracted)
        nc.scalar.activation(out[:, 4:4 + blk], objv, Act.Square,
                             bias=1.0, scale=-1.0)
        # classes: sum_c(oh-cb)^2 = 1 - 2*V[ct,best] + sum_c V^2[c,best].
        # ACT squares the bf16 class block early; host supplies V[ct,n]
        # (exact f32) in row cols 11:16; the +1-per-valid-row lands in the
        # host combine.  Pad rows (zero class data) contribute exactly 0.
        nc.scalar.activation(sqv[:], clsr, Act.Square)
        nc.vector.tensor_reduce(
            b5(sst), sqv[:].rearrange("p (c n cc) -> p c n cc",
                                      n=NUM, cc=CLASSES), Ax.X, Alu.add)
        nc.vector.scalar_tensor_tensor(b5(sst), vctv, -2.0, b5(sst),
                                       Alu.mult, Alu.add)
        nc.vector.tensor_tensor(sst[:], sst[:], t1b[:], Alu.mult)
        nc.vector.tensor_reduce(out[:, 8:8 + blk],
                                b5(sst).rearrange("p c n -> p c n"),
                                Ax.X, Alu.add)

        nc.sync.dma_start(outd[:], out[:])
    nc.finalize()
    return nc


def _prep(x, truth, anchors):
    f32 = np.float32
    x = np.ascontiguousarray(x, f32)
    truth = np.ascontiguousarray(truth, f32)
    anchors = np.asarray(anchors, f32)

    wt, ht = truth[..., 2], truth[..., 3]
    valid = np.cumprod((wt >= 1e-5).astype(f32), axis=1, dtype=f32)
    i = np.clip((truth[..., 0] * f32(W)).astype(np.int32), 0, W - 1)
    j = np.clip((truth[..., 1] * f32(H)).astype(np.int32), 0, H - 1)
    lin = j * W + i                                        # [B,T]
    tx = i.astype(f32) / f32(W)
    ty = j.astype(f32) / f32(H)
    tw = np.exp(wt) * anchors[2 * (NUM - 1)] / f32(W)
    th = np.exp(ht) * anchors[2 * (NUM - 1) + 1] / f32(H)
    at = wt * ht
    scale = (f32(2.0) - at).astype(f32)
    s2 = scale * scale
    txy2 = tx * tx + ty * ty
    ct = np.clip(truth[..., 4].astype(np.int32), 0, CLASSES - 1).astype(f32)

    # gathered rows at the truth cells: [B, T, 16] f32 + [B, T, 100] bf16
    x5 = x.reshape(B, NUM, AL, HW)
    g = x5[np.arange(B)[:, None], :, :, lin]               # [B,T,NUM,AL]
    rows = np.zeros((B, T, 16), f32)
    rows[:, :, 0:5] = g[:, :, :, 2]
    rows[:, :, 5:10] = g[:, :, :, 3]
    rows[:, :, 10] = g[:, :, 0, 4]
    vct = np.take_along_axis(
        g[:, :, :, 5:25],
        np.clip(truth[..., 4].astype(np.int32), 0, CLASSES - 1)[
            :, :, None, None].repeat(NUM, axis=2), axis=3)[..., 0]  # [B,T,5]
    rows[:, :, 11:16] = vct
    crows = np.ascontiguousarray(
        g[:, :, :, 5:25].reshape(B, T, NUM * CLASSES)
    ).astype(ml_dtypes.bfloat16)                       # anchor-major

    # fields: wt ht at tw th s2 txy2*s2 _ ct  (order matches packing below)
    fields = np.stack([wt, ht, at, tw, th, s2, txy2 * s2, valid, ct],
                      axis=-1)                             # [B,T,9]

    vmask = valid.astype(bool)
    nv_core = [int(vmask[BL * c:BL * (c + 1)].sum()) for c in range(NCORES)]
    blk = max(1, -(-max(nv_core) // 128))

    # batch-63 truths with zero area can't set the noobj mask -> prune
    t63 = truth[B - 1]
    live = (t63[:, 2] * t63[:, 3]) > 0.0
    tn = max(1, int(live.sum()))
    t63l = t63[live][:tn]
    bl_ = t63l[:, 0] - f32(0.5) * t63l[:, 2]
    bu_ = t63l[:, 1] - f32(0.5) * t63l[:, 3]
    br_ = t63l[:, 0] + f32(0.5) * t63l[:, 2]
    bd_ = t63l[:, 1] + f32(0.5) * t63l[:, 3]
    hat3 = (t63l[:, 2] * t63l[:, 3]) / f32(3.0)
    tbrow = np.concatenate([bl_, bu_, br_, bd_, hat3]).astype(ml_dtypes.bfloat16)

    xp63 = x[B - 1].reshape(NUM * AL, HW)

    def fold(vec, n):
        v = np.zeros(blk * 128, f32)
        v[:n] = vec
        return v.reshape(blk, 128).T                       # [128, blk]

    in_maps = []
    pads = []
    b63_pos = None
    for c in range(NCORES):
        bs = slice(BL * c, BL * (c + 1))
        cells = slice(CELLS * c, CELLS * (c + 1))
        m = vmask[bs]                                      # [BL, T]
        n = int(m.sum())

        rw = rows[bs][m]                                   # [n, 16]
        pad = np.broadcast_to(rw[0:1] if n else np.zeros((1, 16), f32),
                              (blk * 128 - n, 16))
        rw512 = np.concatenate([rw, np.ascontiguousarray(pad)])
        rw512[n:, 10:16] = 0.0         # pad obj/vct -> exact constants
        xgc = np.ascontiguousarray(
            rw512.reshape(blk, 128, 16).transpose(1, 0, 2).reshape(128, -1))
        cw = crows[bs][m]                                  # [n, 100] bf16
        cpad = np.zeros((blk * 128 - n, 100), ml_dtypes.bfloat16)
        cw512 = np.concatenate([cw, cpad])
        clsc = np.ascontiguousarray(
            cw512.reshape(blk, 128, 100).transpose(1, 0, 2).reshape(128, -1))

        fc = fields[bs][m]                                 # [n, 9]
        tvw = 7 * blk
        tv = np.zeros((128, tvw), f32)
        tv[:, 0:2 * blk:2] = fold(fc[:, 0], n)             # wt (interleaved)
        tv[:, 1:2 * blk:2] = fold(fc[:, 1], n)             # ht
        tv[:, 2 * blk:3 * blk] = fold(fc[:, 2], n)         # at
        tv[:, 3 * blk:5 * blk:2] = fold(fc[:, 3], n)       # tw
        tv[:, 3 * blk + 1:5 * blk:2] = fold(fc[:, 4], n)   # th
        tv[:, 5 * blk:6 * blk] = fold(fc[:, 5], n)         # s2 (packed->vld=1)
        tv[:, 6 * blk:7 * blk] = fold(fc[:, 6], n)         # txy2*s2

        p63 = np.empty((128, 21), f32)
        for an in range(NUM):
            for cc in range(4):
                p63[:, an * 4 + cc] = xp63[AL * an + cc, cells]
        p63[:, 20] = xp63[4, cells]

        pads.append(blk * 128 - n)
        if c == NCORES - 1:
            start = int(m[:BL - 1].sum())
            b63_pos = (start, int(m[BL - 1].sum()))

        in_maps.append({
            "pf32": np.ascontiguousarray(
                np.concatenate([p63, tv, xgc], axis=1)),
            "pbf": np.ascontiguousarray(np.concatenate(
                [np.broadcast_to(tbrow, (128, 5 * tn)), clsc], axis=1)),
        })
    return in_maps, blk, tn, (b63_pos, pads)


def _combine(results, blk, b63_info):
    b63_pos, pads = b63_info
    npad = float(sum(pads))
    obj = sum(float(r["out"][:, 4:4 + blk].sum(dtype=np.float64))
              for r in results) - npad
    cls = sum(float(r["out"][:, 8:8 + blk].sum(dtype=np.float64))
              for r in results) + (NCORES * blk * 128 - npad)
    noobj = sum(float(r["out"][:, 12].sum(dtype=np.float64)) for r in results)
    sq = sum(float(r["out"][:, 13].sum(dtype=np.float64)) for r in results)
    xy = sum(float(r["out"][:, 14].sum(dtype=np.float64)) for r in results)
    r7 = results[NCORES - 1]["out"]
    start, cnt = b63_pos
    coord63 = 0.0
    for idx in range(start, start + cnt):
        coord63 += float(r7[idx % 128, idx // 128])
    warm = 0.01 * (sq - xy + 0.5 * NUM * HW)
    return np.float32(obj + noobj + warm + coord63 + cls)


def kernel(x, truth, anchors, **_):
    in_maps, blk, tn, b63_pos = _prep(x, truth, anchors)
    key = (blk, tn)
    if key not in _CACHED:
        _CACHED[key] = _build_program(blk, tn)
    nc = _CACHED[key]
    res = run_bass_kernel_spmd(nc, in_maps, list(range(NCORES)))
    return _combine(res.results, blk, b63_pos)


# revision 5
# speedup vs baseline: 1.0258x; 1.0258x over previous
"""YOLOv2-style loss (nn_CostYoloV2) on 8 Trainium2 NeuronCores — v2.

Data-parallel over batch (8 batches/core); the noobj term (batch 63 only)
is sharded over the 1024 spatial cells (128 cells/core).

Layout strategy: the truth-cell gather indices are pure host data
(computed from `truth` alone), so the host packs the valid (batch, truth)
rows per core into a [128 part, BLK block, 128 ch] tile and the device
consumes them directly — measured on-device alternatives (streaming all
23 used channels of x + gpsimd indirect_copy + PE transpose: ~26us of DMA
at the 2-queue HWDGE ceiling; swdge dma_gather: ~15us of descriptor
generation; trn2's Pool engine has no elementwise ALU) are all slower.
Rows with valid=0 and all-zero truth slots are pruned host-side (their
loss contributions are provably zero), shrinking both DMA and DVE work.

Per-(b,t) row layout (128 f32): [w x5 anchors | h x5 | obj0 | pad x5 |
cls x20 anchor-innermost | pad x12].  Per-(b,t) scalars ride in a side
tile.  Engine split: DVE runs both the per-(b,t) iou/argmax/coord/class
chain and the noobj mask chain, ACT does relus/squares/accumulations,
PE broadcasts the truth-corner constants across partitions.  Scalar
partials return per core and are combined on the host (the "all-reduce"
of the loss terms).
"""
import numpy as np
import ml_dtypes
from contextlib import ExitStack

import concourse.bass as bass
import concourse.bacc as bacc
import concourse.mybir as mybir
import concourse.tile as tile
from concourse.bass_utils import run_bass_kernel_spmd

F32 = mybir.dt.float32
BF16 = mybir.dt.bfloat16
Alu = mybir.AluOpType
Ax = mybir.AxisListType
Act = mybir.ActivationFunctionType

B, NUM, CLASSES, AL = 64, 5, 20, 25
H = W = 32
HW = H * W
T = 50
NCORES = 8
BL = B // NCORES       # local batches per core
ELEM = 128             # row width (channels, padded)
CELLS = HW // NCORES   # 128 noobj cells per core

_CACHED = {}


def _build_program(blk, tn):
    """blk: number of 128-row blocks of packed (b,t) rows; tn: live truths
    of batch 63 for the noobj term."""
    nc = bacc.Bacc(enable_partition_id=False)
    tvw = 7 * blk
    o_pp = 0                           # pf32 = p63 | tv | xgf
    o_tv = 21
    o_xg = o_tv + tvw
    nf = o_xg + blk * 16
    nb = 5 * tn + blk * CLASSES * NUM  # pbf = tbs | cls
    pf32 = nc.declare_dram_parameter("pf32", [128, nf], F32, isOutput=False)
    pbf = nc.declare_dram_parameter("pbf", [128, nb], BF16, isOutput=False)
    outd = nc.declare_dram_parameter("out", [128, 16], F32, isOutput=True)

    o_wtht, o_at = 0, 2 * blk
    o_twth, o_s2v, o_txys = 3 * blk, 5 * blk, 6 * blk

    with tile.TileContext(nc) as tc, ExitStack() as ctx:
        pool = ctx.enter_context(tc.tile_pool(name="p", bufs=1))

        # -------- input DMAs ------------------------------------------
        itf = pool.tile([128, nf], F32)
        itb = pool.tile([128, nb], BF16)
        nc.sync.dma_start(itb[:, 0:5 * tn], pbf[:, 0:5 * tn])  # tbs
        nc.scalar.dma_start(itf[:, 0:o_xg], pf32[:, 0:o_xg])   # p63|tv
        nc.scalar.dma_start(itf[:, o_xg:], pf32[:, o_xg:])     # xgf
        nc.scalar.dma_start(itb[:, 5 * tn:], pbf[:, 5 * tn:])  # cls
        xt = itf[:, o_xg:o_xg + blk * 16]
        tbs = itb[:, 0:5 * tn]
        clsr = itb[:, 5 * tn:]
        pp = itf[:, o_pp:o_pp + 21]
        tv = itf[:, o_tv:o_tv + tvw]

        out = pool.tile([128, 16], F32)
        nc.gpsimd.memset(out[:], 0.0)

        def tb_b(col):  # truth block [128,tn] -> [128,5,tn] (bcast anchors)
            return tbs[:, col * tn:(col + 1) * tn].rearrange(
                "p (o f) -> p o f", o=1).broadcast_to([128, NUM, tn])

        # ---------------- noobj (batch 63, this core's cells) -------------
        pv = pp[:, 0:20].rearrange("p (n c) -> p n c", c=4)
        xc, yc, wc, hc = pv[:, :, 0], pv[:, :, 1], pv[:, :, 2], pv[:, :, 3]
        corn = pool.tile([128, 25], F32)
        lo = corn[:, 0:10].rearrange("p (g n) -> p n g", g=2)    # al|au
        hi = corn[:, 10:20].rearrange("p (g n) -> p n g", g=2)   # ar|ad
        hap3 = corn[:, 20:25]
        pxy = pv[:, :, 0:2]
        pwh = pv[:, :, 2:4]
        nc.vector.scalar_tensor_tensor(lo, pwh, -0.5, pxy, Alu.mult, Alu.add)
        nc.vector.scalar_tensor_tensor(hi, pwh, 0.5, pxy, Alu.mult, Alu.add)
        # hap/1.5 = wc*hc/3  (mask test: iou>0.5 <=> inter-hat/1.5 > hap/1.5)
        nc.vector.scalar_tensor_tensor(hap3, wc, 1.0 / 3.0, hc,
                                       Alu.mult, Alu.mult)

        al, au = corn[:, 0:5], corn[:, 5:10]
        ar, ad = corn[:, 10:15], corn[:, 15:20]

        def cn_b(apv):  # corner col [128,5] -> [128,5,tn]
            return apv.broadcast_to([128, NUM, tn])

        u1 = pool.tile([128, NUM * tn], F32)
        u2 = pool.tile([128, NUM * tn], F32)
        iw = pool.tile([128, NUM * tn], F32)
        ih = pool.tile([128, NUM * tn], F32)

        def w3(tl):
            return tl[:].rearrange("p (n t) -> p n t", t=tn)

        nc.vector.tensor_tensor(w3(u1), tb_b(2), cn_b(ar), Alu.min)
        nc.vector.tensor_tensor(w3(u2), tb_b(0), cn_b(al), Alu.max)
        nc.vector.tensor_tensor(iw[:], u1[:], u2[:], Alu.subtract)
        nc.vector.tensor_tensor(w3(u1), tb_b(3), cn_b(ad), Alu.min)
        nc.vector.tensor_tensor(w3(u2), tb_b(1), cn_b(au), Alu.max)
        nc.vector.tensor_tensor(ih[:], u1[:], u2[:], Alu.subtract)
        nc.scalar.activation(ih[:], ih[:], Act.Relu)
        nc.vector.tensor_tensor(iw[:], iw[:], ih[:], Alu.mult)        # inter
        # g = inter - hat/1.5 ; mask_n = (max_t g) > hap/1.5
        nc.vector.tensor_tensor(w3(iw), w3(iw), tb_b(4), Alu.subtract)
        anyt = pool.tile([128, 6], F32)
        nc.vector.tensor_reduce(anyt[:, 0:5], w3(iw), Ax.X, Alu.max)
        # unmasked-anchor count in one op: [mx <= hap3] summed via accum
        nc.vector.scalar_tensor_tensor(anyt[:, 0:5], anyt[:, 0:5], 1.0,
                                       hap3, Alu.mult, Alu.is_le,
                                       accum_out=anyt[:, 5:6])
        oq2 = pool.tile([128, 1], F32)
        nc.scalar.square(oq2[:], pp[:, 20:21])
        nc.vector.tensor_tensor(out[:, 12:13], anyt[:, 5:6], oq2[:],
                                Alu.mult)

        # ---------------- warm coords (batch 63 cells) --------------------
        scr = pool.tile([128, 20], F32)
        nc.scalar.activation(scr[:], pp[:, 0:20], Act.Square,
                             accum_out=out[:, 13:14])
        xysc = pool.tile([128, 10], F32)
        nc.scalar.activation(
            xysc[:].rearrange("p (n g) -> p n g", g=2), pv[:, :, 0:2],
            Act.Copy, accum_out=out[:, 14:15])

        # ---------------- per-(b,t) losses — DVE --------------------------
        x3 = xt.rearrange("p (c e) -> p c e", e=16)
        wv = x3[:, :, 0:5]
        hv = x3[:, :, 5:10]
        objv = x3[:, :, 10]                          # [128,blk]
        wh4 = x3[:, :, 0:10].rearrange("p c (g n) -> p c g n", g=2)
        vctv = x3[:, :, 11:16]                       # [128,blk,5]

        wtht_b = tv[:, o_wtht:o_wtht + 2 * blk].rearrange(
            "p (c g o) -> p c g o", g=2, o=1).broadcast_to([128, blk, 2, NUM])
        at_b = tv[:, o_at:o_at + blk].broadcast_to([128, blk, NUM])
        twth = tv[:, o_twth:o_twth + 2 * blk]
        s2v = tv[:, o_s2v:o_s2v + blk]
        txys = tv[:, o_txys:o_txys + blk]               # txy2*s2 (host-folded)

        t1 = pool.tile([128, blk * NUM], F32)
        t2 = pool.tile([128, blk * 2 * NUM], F32)
        t3 = pool.tile([128, blk * NUM], F32)
        t4 = pool.tile([128, blk * NUM], F32)
        mx = pool.tile([128, blk], F32)
        wbhb = pool.tile([128, blk * 2], F32)
        cdf = pool.tile([128, blk], F32)
        t6 = pool.tile([128, blk * 2 * NUM], F32)
        t1b = pool.tile([128, blk * NUM], BF16)
        sqv = pool.tile([128, blk * CLASSES * NUM], F32)
        sst = pool.tile([128, blk * NUM], F32)

        def b5(tl):
            return tl[:].rearrange("p (c n) -> p c n", n=NUM)

        def b20(tl):
            return tl[:].rearrange("p (c e) -> p c e", e=CLASSES)

        t12 = t2[:].rearrange("p (c g n) -> p c g n", g=2, n=NUM)
        nc.vector.tensor_tensor(t12, wh4, wtht_b, Alu.min)
        nc.vector.tensor_tensor(b5(t3), t12[:, :, 0], t12[:, :, 1],
                                Alu.mult)                          # inter
        # argmax_n inter/union == argmax_n inter/(A_n+B): x/(S-x) is
        # monotone in x/S, so the -inter term of the union drops out.
        nc.vector.tensor_tensor(b5(t4), wv, hv, Alu.mult)
        nc.vector.tensor_tensor(b5(t4), b5(t4), at_b, Alu.add)     # A+B
        nc.vector.reciprocal(t4[:], t4[:])
        nc.vector.tensor_tensor(t3[:], t3[:], t4[:], Alu.mult)     # score
        nc.vector.tensor_reduce(mx[:], b5(t3), Ax.X, Alu.max)
        nc.vector.tensor_tensor(b5(t1b), b5(t3),
                                mx[:].broadcast_to([128, blk, NUM]), Alu.is_ge)
        m4 = t1b[:].rearrange("p (c o n) -> p c o n", o=1, n=NUM)
        t6wh = t6[:, 0:blk * 2 * NUM].rearrange(
            "p (c g n) -> p c g n", g=2, n=NUM)
        nc.vector.tensor_tensor(
            t6wh, wh4, m4.broadcast_to([128, blk, 2, NUM]), Alu.mult)
        nc.vector.tensor_reduce(
            wbhb[:].rearrange("p (c g) -> p c g", g=2), t6wh, Ax.X, Alu.add)
        # coords: ((tw-wb)^2 + (th-hb)^2)*s2 + txy2*s2
        nc.vector.tensor_tensor(wbhb[:], twth, wbhb[:], Alu.subtract)
        nc.vector.tensor_tensor(wbhb[:], wbhb[:], wbhb[:], Alu.mult)
        nc.vector.tensor_reduce(cdf[:], wbhb[:].rearrange(
            "p (c g) -> p c g", g=2), Ax.X, Alu.add)
        nc.vector.tensor_tensor(cdf[:], cdf[:], s2v, Alu.mult)
        nc.vector.tensor_tensor(out[:, 0:blk], cdf[:], txys, Alu.add)
        # obj: (1-obj)^2; pad rows contribute exactly 1 (host-subtracted)
        nc.scalar.activation(out[:, 4:4 + blk], objv, Act.Square,
                             bias=1.0, scale=-1.0)
        # classes: sum_c(oh-cb)^2 = 1 - 2*V[ct,best] + sum_c V^2[c,best].
        # ACT squares the bf16 class block early; host supplies V[ct,n]
        # (exact f32) in row cols 11:16; the +1-per-valid-row lands in the
        # host combine.  Pad rows (zero class data) contribute exactly 0.
        nc.scalar.activation(sqv[:], clsr, Act.Square)
        nc.vector.tensor_reduce(
            b5(sst), sqv[:].rearrange("p (c n cc) -> p c n cc",
                                      n=NUM, cc=CLASSES), Ax.X, Alu.add)
        nc.vector.scalar_tensor_tensor(b5(sst), vctv, -2.0, b5(sst),
                                       Alu.mult, Alu.add)
        nc.vector.tensor_tensor(sst[:], sst[:], t1b[:], Alu.mult)
        nc.vector.tensor_reduce(out[:, 8:8 + blk],
                                b5(sst).rearrange("p c n -> p c n"),
                                Ax.X, Alu.add)

        nc.sync.dma_start(outd[:], out[:])
    nc.finalize()
    return nc


def _prep(x, truth, anchors):
    f32 = np.float32
    x = np.ascontiguousarray(x, f32)
    truth = np.ascontiguousarray(truth, f32)
    anchors = np.asarray(anchors, f32)

    wt, ht = truth[..., 2], truth[..., 3]
    valid = np.cumprod((wt >= 1e-5).astype(f32), axis=1, dtype=f32)
    i = np.clip((truth[..., 0] * f32(W)).astype(np.int32), 0, W - 1)
    j = np.clip((truth[..., 1] * f32(H)).astype(np.int32), 0, H - 1)
    lin = j * W + i                                        # [B,T]
    tx = i.astype(f32) / f32(W)
    ty = j.astype(f32) / f32(H)
    tw = np.exp(wt) * anchors[2 * (NUM - 1)] / f32(W)
    th = np.exp(ht) * anchors[2 * (NUM - 1) + 1] / f32(H)
    at = wt * ht
    scale = (f32(2.0) - at).astype(f32)
    s2 = scale * scale
    txy2 = tx * tx + ty * ty
    ct = np.clip(truth[..., 4].astype(np.int32), 0, CLASSES - 1).astype(f32)

    # gathered rows at the truth cells: [B, T, 16] f32 + [B, T, 100] bf16
    x5 = x.reshape(B, NUM, AL, HW)
    g = x5[np.arange(B)[:, None], :, :, lin]               # [B,T,NUM,AL]
    rows = np.zeros((B, T, 16), f32)
    rows[:, :, 0:5] = g[:, :, :, 2]
    rows[:, :, 5:10] = g[:, :, :, 3]
    rows[:, :, 10] = g[:, :, 0, 4]
    vct = np.take_along_axis(
        g[:, :, :, 5:25],
        np.clip(truth[..., 4].astype(np.int32), 0, CLASSES - 1)[
            :, :, None, None].repeat(NUM, axis=2), axis=3)[..., 0]  # [B,T,5]
    rows[:, :, 11:16] = vct
    crows = np.ascontiguousarray(
        g[:, :, :, 5:25].reshape(B, T, NUM * CLASSES)
    ).astype(ml_dtypes.bfloat16)                       # anchor-major

    # fields: wt ht at tw th s2 txy2*s2 _ ct  (order matches packing below)
    fields = np.stack([wt, ht, at, tw, th, s2, txy2 * s2, valid, ct],
                      axis=-1)                             # [B,T,9]

    vmask = valid.astype(bool)
    nv_core = [int(vmask[BL * c:BL * (c + 1)].sum()) for c in range(NCORES)]
    blk = max(1, -(-max(nv_core) // 128))

    # batch-63 truths with zero area can't set the noobj mask -> prune
    t63 = truth[B - 1]
    live = (t63[:, 2] * t63[:, 3]) > 0.0
    tn = max(1, int(live.sum()))
    t63l = t63[live][:tn]
    bl_ = t63l[:, 0] - f32(0.5) * t63l[:, 2]
    bu_ = t63l[:, 1] - f32(0.5) * t63l[:, 3]
    br_ = t63l[:, 0] + f32(0.5) * t63l[:, 2]
    bd_ = t63l[:, 1] + f32(0.5) * t63l[:, 3]
    hat3 = (t63l[:, 2] * t63l[:, 3]) / f32(3.0)
    tbrow = np.concatenate([bl_, bu_, br_, bd_, hat3]).astype(ml_dtypes.bfloat16)

    xp63 = x[B - 1].reshape(NUM * AL, HW)

    def fold(vec, n):
        v = np.zeros(blk * 128, f32)
        v[:n] = vec
        return v.reshape(blk, 128).T                       # [128, blk]

    in_maps = []
    pads = []
    b63_pos = None
    for c in range(NCORES):
        bs = slice(BL * c, BL * (c + 1))
        cells = slice(CELLS * c, CELLS * (c + 1))
        m = vmask[bs]                                      # [BL, T]
        n = int(m.sum())

        rw = rows[bs][m]                                   # [n, 16]
        pad = np.broadcast_to(rw[0:1] if n else np.zeros((1, 16), f32),
                              (blk * 128 - n, 16))
        rw512 = np.concatenate([rw, np.ascontiguousarray(pad)])
        rw512[n:, 10:16] = 0.0         # pad obj/vct -> exact constants
        xgc = np.ascontiguousarray(
            rw512.reshape(blk, 128, 16).transpose(1, 0, 2).reshape(128, -1))
        cw = crows[bs][m]                                  # [n, 100] bf16
        cpad = np.zeros((blk * 128 - n, 100), ml_dtypes.bfloat16)
        cw512 = np.concatenate([cw, cpad])
        clsc = np.ascontiguousarray(
            cw512.reshape(blk, 128, 100).transpose(1, 0, 2).reshape(128, -1))

        fc = fields[bs][m]                                 # [n, 9]
        tvw = 7 * blk
        tv = np.zeros((128, tvw), f32)
        tv[:, 0:2 * blk:2] = fold(fc[:, 0], n)             # wt (interleaved)
        tv[:, 1:2 * blk:2] = fold(fc[:, 1], n)             # ht
        tv[:, 2 * blk:3 * blk] = fold(fc[:, 2], n)         # at
        tv[:, 3 * blk:5 * blk:2] = fold(fc[:, 3], n)       # tw
        tv[:, 3 * blk + 1:5 * blk:2] = fold(fc[:, 4], n)   # th
        tv[:, 5 * blk:6 * blk] = fold(fc[:, 5], n)         # s2 (packed->vld=1)
        tv[:, 6 * blk:7 * blk] = fold(fc[:, 6], n)         # txy2*s2

        p63 = np.empty((128, 21), f32)
        for an in range(NUM):
            for cc in range(4):
                p63[:, an * 4 + cc] = xp63[AL * an + cc, cells]
        p63[:, 20] = xp63[4, cells]

        pads.append(blk * 128 - n)
        if c == NCORES - 1:
            start = int(m[:BL - 1].sum())
            b63_pos = (start, int(m[BL - 1].sum()))

        in_maps.append({
            "pf32": np.ascontiguousarray(
                np.concatenate([p63, tv, xgc], axis=1)),
            "pbf": np.ascontiguousarray(np.concatenate(
                [np.broadcast_to(tbrow, (128, 5 * tn)), clsc], axis=1)),
        })
    return in_maps, blk, tn, (b63_pos, pads)


def _combine(results, blk, b63_info):
    b63_pos, pads = b63_info
    npad = float(sum(pads))
    obj = sum(float(r["out"][:, 4:4 + blk].sum(dtype=np.float64))
              for r in results) - npad
    cls = sum(float(r["out"][:, 8:8 + blk].sum(dtype=np.float64))
              for r in results) + (NCORES * blk * 128 - npad)
    noobj = sum(float(r["out"][:, 12].sum(dtype=np.float64)) for r in results)
    sq = sum(float(r["out"][:, 13].sum(dtype=np.float64)) for r in results)
    xy = sum(float(r["out"][:, 14].sum(dtype=np.float64)) for r in results)
    r7 = results[NCORES - 1]["out"]
    start, cnt = b63_pos
    coord63 = 0.0
    for idx in range(start, start + cnt):
        coord63 += float(r7[idx % 128, idx // 128])
    warm = 0.01 * (sq - xy + 0.5 * NUM * HW)
    return np.float32(obj + noobj + warm + coord63 + cls)


def kernel(x, truth, anchors, **_):
    in_maps, blk, tn, b63_pos = _prep(x, truth, anchors)
    key = (blk, tn)
    if key not in _CACHED:
        _CACHED[key] = _build_program(blk, tn)
    nc = _CACHED[key]
    res = run_bass_kernel_spmd(nc, in_maps, list(range(NCORES)))
    return _combine(res.results, blk, b63_pos)


# revision 6
# speedup vs baseline: 1.0260x; 1.0003x over previous
"""YOLOv2-style loss (nn_CostYoloV2) on 8 Trainium2 NeuronCores — v2.

Data-parallel over batch (8 batches/core); the noobj term (batch 63 only)
is sharded over the 1024 spatial cells (128 cells/core).

Layout strategy: the truth-cell gather indices are pure host data
(computed from `truth` alone), so the host packs the valid (batch, truth)
rows per core into a [128 part, BLK block, 128 ch] tile and the device
consumes them directly — measured on-device alternatives (streaming all
23 used channels of x + gpsimd indirect_copy + PE transpose: ~26us of DMA
at the 2-queue HWDGE ceiling; swdge dma_gather: ~15us of descriptor
generation; trn2's Pool engine has no elementwise ALU) are all slower.
Rows with valid=0 and all-zero truth slots are pruned host-side (their
loss contributions are provably zero), shrinking both DMA and DVE work.

Per-(b,t) row layout (128 f32): [w x5 anchors | h x5 | obj0 | pad x5 |
cls x20 anchor-innermost | pad x12].  Per-(b,t) scalars ride in a side
tile.  Engine split: DVE runs both the per-(b,t) iou/argmax/coord/class
chain and the noobj mask chain, ACT does relus/squares/accumulations,
PE broadcasts the truth-corner constants across partitions.  Scalar
partials return per core and are combined on the host (the "all-reduce"
of the loss terms).
"""
import numpy as np
import ml_dtypes
from contextlib import ExitStack

import concourse.bass as bass
import concourse.bacc as bacc
import concourse.mybir as mybir
import concourse.tile as tile
from concourse.bass_utils import run_bass_kernel_spmd

F32 = mybir.dt.float32
BF16 = mybir.dt.bfloat16
Alu = mybir.AluOpType
Ax = mybir.AxisListType
Act = mybir.ActivationFunctionType

B, NUM, CLASSES, AL = 64, 5, 20, 25
H = W = 32
HW = H * W
T = 50
NCORES = 8
BL = B // NCORES       # local batches per core
ELEM = 128             # row width (channels, padded)
CELLS = HW // NCORES   # 128 noobj cells per core

_CACHED = {}


def _build_program(blk, tn):
    """blk: number of 128-row blocks of packed (b,t) rows; tn: live truths
    of batch 63 for the noobj term."""
    nc = bacc.Bacc(enable_partition_id=False)
    tvw = 7 * blk
    o_pp = 0                           # pf32 = p63 | tv | xgf
    o_tv = 21
    o_xg = o_tv + tvw
    nf = o_xg + blk * 16
    nb = 5 * tn + blk * CLASSES * NUM  # pbf = tbs | cls
    pf32 = nc.declare_dram_parameter("pf32", [128, nf], F32, isOutput=False)
    pbf = nc.declare_dram_parameter("pbf", [128, nb], BF16, isOutput=False)
    outd = nc.declare_dram_parameter("out", [128, 16], F32, isOutput=True)

    o_wtht, o_at = 0, 2 * blk
    o_twth, o_s2v, o_txys = 3 * blk, 5 * blk, 6 * blk

    with tile.TileContext(nc) as tc, ExitStack() as ctx:
        pool = ctx.enter_context(tc.tile_pool(name="p", bufs=1))

        # -------- input DMAs ------------------------------------------
        itf = pool.tile([128, nf], F32)
        itb = pool.tile([128, nb], BF16)
        nc.sync.dma_start(itb[:, 0:5 * tn], pbf[:, 0:5 * tn])  # tbs
        nc.scalar.dma_start(itf[:], pf32[:])                   # p63|tv|xgf
        nc.scalar.dma_start(itb[:, 5 * tn:], pbf[:, 5 * tn:])  # cls
        xt = itf[:, o_xg:o_xg + blk * 16]
        tbs = itb[:, 0:5 * tn]
        clsr = itb[:, 5 * tn:]
        pp = itf[:, o_pp:o_pp + 21]
        tv = itf[:, o_tv:o_tv + tvw]

        out = pool.tile([128, 16], F32)
        nc.gpsimd.memset(out[:], 0.0)

        def tb_b(col):  # truth block [128,tn] -> [128,5,tn] (bcast anchors)
            return tbs[:, col * tn:(col + 1) * tn].rearrange(
                "p (o f) -> p o f", o=1).broadcast_to([128, NUM, tn])

        # ---------------- noobj (batch 63, this core's cells) -------------
        pv = pp[:, 0:20].rearrange("p (n c) -> p n c", c=4)
        xc, yc, wc, hc = pv[:, :, 0], pv[:, :, 1], pv[:, :, 2], pv[:, :, 3]
        corn = pool.tile([128, 25], F32)
        lo = corn[:, 0:10].rearrange("p (g n) -> p n g", g=2)    # al|au
        hi = corn[:, 10:20].rearrange("p (g n) -> p n g", g=2)   # ar|ad
        hap3 = corn[:, 20:25]
        pxy = pv[:, :, 0:2]
        pwh = pv[:, :, 2:4]
        nc.vector.scalar_tensor_tensor(lo, pwh, -0.5, pxy, Alu.mult, Alu.add)
        nc.vector.scalar_tensor_tensor(hi, pwh, 0.5, pxy, Alu.mult, Alu.add)
        # hap/1.5 = wc*hc/3  (mask test: iou>0.5 <=> inter-hat/1.5 > hap/1.5)
        nc.vector.scalar_tensor_tensor(hap3, wc, 1.0 / 3.0, hc,
                                       Alu.mult, Alu.mult)

        al, au = corn[:, 0:5], corn[:, 5:10]
        ar, ad = corn[:, 10:15], corn[:, 15:20]

        def cn_b(apv):  # corner col [128,5] -> [128,5,tn]
            return apv.broadcast_to([128, NUM, tn])

        u1 = pool.tile([128, NUM * tn], F32)
        u2 = pool.tile([128, NUM * tn], F32)
        iw = pool.tile([128, NUM * tn], F32)
        ih = pool.tile([128, NUM * tn], F32)

        def w3(tl):
            return tl[:].rearrange("p (n t) -> p n t", t=tn)

        nc.vector.tensor_tensor(w3(u1), tb_b(2), cn_b(ar), Alu.min)
        nc.vector.tensor_tensor(w3(u2), tb_b(0), cn_b(al), Alu.max)
        nc.vector.tensor_tensor(iw[:], u1[:], u2[:], Alu.subtract)
        nc.vector.tensor_tensor(w3(u1), tb_b(3), cn_b(ad), Alu.min)
        nc.vector.tensor_tensor(w3(u2), tb_b(1), cn_b(au), Alu.max)
        nc.vector.tensor_tensor(ih[:], u1[:], u2[:], Alu.subtract)
        nc.scalar.activation(ih[:], ih[:], Act.Relu)
        nc.vector.tensor_tensor(iw[:], iw[:], ih[:], Alu.mult)        # inter
        # g = inter - hat/1.5 ; mask_n = (max_t g) > hap/1.5
        nc.vector.tensor_tensor(w3(iw), w3(iw), tb_b(4), Alu.subtract)
        anyt = pool.tile([128, 6], F32)
        nc.vector.tensor_reduce(anyt[:, 0:5], w3(iw), Ax.X, Alu.max)
        # unmasked-anchor count in one op: [mx <= hap3] summed via accum
        nc.vector.scalar_tensor_tensor(anyt[:, 0:5], anyt[:, 0:5], 1.0,
                                       hap3, Alu.mult, Alu.is_le,
                                       accum_out=anyt[:, 5:6])
        oq2 = pool.tile([128, 1], F32)
        nc.scalar.square(oq2[:], pp[:, 20:21])
        nc.vector.tensor_tensor(out[:, 12:13], anyt[:, 5:6], oq2[:],
                                Alu.mult)

        # ---------------- warm coords (batch 63 cells) --------------------
        scr = pool.tile([128, 20], F32)
        nc.scalar.activation(scr[:], pp[:, 0:20], Act.Square,
                             accum_out=out[:, 13:14])
        xysc = pool.tile([128, 10], F32)
        nc.scalar.activation(
            xysc[:].rearrange("p (n g) -> p n g", g=2), pv[:, :, 0:2],
            Act.Copy, accum_out=out[:, 14:15])

        # ---------------- per-(b,t) losses — DVE --------------------------
        x3 = xt.rearrange("p (c e) -> p c e", e=16)
        wv = x3[:, :, 0:5]
        hv = x3[:, :, 5:10]
        objv = x3[:, :, 10]                          # [128,blk]
        wh4 = x3[:, :, 0:10].rearrange("p c (g n) -> p c g n", g=2)
        vctv = x3[:, :, 11:16]                       # [128,blk,5]

        wtht_b = tv[:, o_wtht:o_wtht + 2 * blk].rearrange(
            "p (c g o) -> p c g o", g=2, o=1).broadcast_to([128, blk, 2, NUM])
        at_b = tv[:, o_at:o_at + blk].broadcast_to([128, blk, NUM])
        twth = tv[:, o_twth:o_twth + 2 * blk]
        s2v = tv[:, o_s2v:o_s2v + blk]
        txys = tv[:, o_txys:o_txys + blk]               # txy2*s2 (host-folded)

        t1 = pool.tile([128, blk * NUM], F32)
        t2 = pool.tile([128, blk * 2 * NUM], F32)
        t3 = pool.tile([128, blk * NUM], F32)
        t4 = pool.tile([128, blk * NUM], F32)
        mx = pool.tile([128, blk], F32)
        wbhb = pool.tile([128, blk * 2], F32)
        cdf = pool.tile([128, blk], F32)
        t6 = pool.tile([128, blk * 2 * NUM], F32)
        t1b = pool.tile([128, blk * NUM], BF16)
        sqv = pool.tile([128, blk * CLASSES * NUM], F32)
        sst = pool.tile([128, blk * NUM], F32)

        def b5(tl):
            return tl[:].rearrange("p (c n) -> p c n", n=NUM)

        def b20(tl):
            return tl[:].rearrange("p (c e) -> p c e", e=CLASSES)

        t12 = t2[:].rearrange("p (c g n) -> p c g n", g=2, n=NUM)
        nc.vector.tensor_tensor(t12, wh4, wtht_b, Alu.min)
        nc.vector.tensor_tensor(b5(t3), t12[:, :, 0], t12[:, :, 1],
                                Alu.mult)                          # inter
        # argmax_n inter/union == argmax_n inter/(A_n+B): x/(S-x) is
        # monotone in x/S, so the -inter term of the union drops out.
        nc.vector.tensor_tensor(b5(t4), wv, hv, Alu.mult)
        nc.vector.tensor_tensor(b5(t4), b5(t4), at_b, Alu.add)     # A+B
        nc.vector.reciprocal(t4[:], t4[:])
        nc.vector.tensor_tensor(t3[:], t3[:], t4[:], Alu.mult)     # score
        nc.vector.tensor_reduce(mx[:], b5(t3), Ax.X, Alu.max)
        nc.vector.tensor_tensor(b5(t1b), b5(t3),
                                mx[:].broadcast_to([128, blk, NUM]), Alu.is_ge)
        m4 = t1b[:].rearrange("p (c o n) -> p c o n", o=1, n=NUM)
        t6wh = t6[:, 0:blk * 2 * NUM].rearrange(
            "p (c g n) -> p c g n", g=2, n=NUM)
        nc.vector.tensor_tensor(
            t6wh, wh4, m4.broadcast_to([128, blk, 2, NUM]), Alu.mult)
        nc.vector.tensor_reduce(
            wbhb[:].rearrange("p (c g) -> p c g", g=2), t6wh, Ax.X, Alu.add)
        # coords: ((tw-wb)^2 + (th-hb)^2)*s2 + txy2*s2
        nc.vector.tensor_tensor(wbhb[:], twth, wbhb[:], Alu.subtract)
        nc.vector.tensor_tensor(wbhb[:], wbhb[:], wbhb[:], Alu.mult)
        nc.vector.tensor_reduce(cdf[:], wbhb[:].rearrange(
            "p (c g) -> p c g", g=2), Ax.X, Alu.add)
        nc.vector.tensor_tensor(cdf[:], cdf[:], s2v, Alu.mult)
        nc.vector.tensor_tensor(out[:, 0:blk], cdf[:], txys, Alu.add)
        # obj: (1-obj)^2; pad rows contribute exactly 1 (host-subtracted)
        nc.scalar.activation(out[:, 4:4 + blk], objv, Act.Square,
                             bias=1.0, scale=-1.0)
        # classes: sum_c(oh-cb)^2 = 1 - 2*V[ct,best] + sum_c V^2[c,best].
        # ACT squares the bf16 class block early; host supplies V[ct,n]
        # (exact f32) in row cols 11:16; the +1-per-valid-row lands in the
        # host combine.  Pad rows (zero class data) contribute exactly 0.
        nc.scalar.activation(sqv[:], clsr, Act.Square)
        nc.vector.tensor_reduce(
            b5(sst), sqv[:].rearrange("p (c n cc) -> p c n cc",
                                      n=NUM, cc=CLASSES), Ax.X, Alu.add)
        nc.vector.scalar_tensor_tensor(b5(sst), vctv, -2.0, b5(sst),
                                       Alu.mult, Alu.add)
        nc.vector.tensor_tensor(sst[:], sst[:], t1b[:], Alu.mult)
        nc.vector.tensor_reduce(out[:, 8:8 + blk],
                                b5(sst).rearrange("p c n -> p c n"),
                                Ax.X, Alu.add)

        nc.sync.dma_start(outd[:], out[:])
    nc.finalize()
    return nc


def _prep(x, truth, anchors):
    f32 = np.float32
    x = np.ascontiguousarray(x, f32)
    truth = np.ascontiguousarray(truth, f32)
    anchors = np.asarray(anchors, f32)

    wt, ht = truth[..., 2], truth[..., 3]
    valid = np.cumprod((wt >= 1e-5).astype(f32), axis=1, dtype=f32)
    i = np.clip((truth[..., 0] * f32(W)).astype(np.int32), 0, W - 1)
    j = np.clip((truth[..., 1] * f32(H)).astype(np.int32), 0, H - 1)
    lin = j * W + i                                        # [B,T]
    tx = i.astype(f32) / f32(W)
    ty = j.astype(f32) / f32(H)
    tw = np.exp(wt) * anchors[2 * (NUM - 1)] / f32(W)
    th = np.exp(ht) * anchors[2 * (NUM - 1) + 1] / f32(H)
    at = wt * ht
    scale = (f32(2.0) - at).astype(f32)
    s2 = scale * scale
    txy2 = tx * tx + ty * ty
    ct = np.clip(truth[..., 4].astype(np.int32), 0, CLASSES - 1).astype(f32)

    # gathered rows at the truth cells: [B, T, 16] f32 + [B, T, 100] bf16
    x5 = x.reshape(B, NUM, AL, HW)
    g = x5[np.arange(B)[:, None], :, :, lin]               # [B,T,NUM,AL]
    rows = np.zeros((B, T, 16), f32)
    rows[:, :, 0:5] = g[:, :, :, 2]
    rows[:, :, 5:10] = g[:, :, :, 3]
    rows[:, :, 10] = g[:, :, 0, 4]
    vct = np.take_along_axis(
        g[:, :, :, 5:25],
        np.clip(truth[..., 4].astype(np.int32), 0, CLASSES - 1)[
            :, :, None, None].repeat(NUM, axis=2), axis=3)[..., 0]  # [B,T,5]
    rows[:, :, 11:16] = vct
    crows = np.ascontiguousarray(
        g[:, :, :, 5:25].reshape(B, T, NUM * CLASSES)
    ).astype(ml_dtypes.bfloat16)                       # anchor-major

    # fields: wt ht at tw th s2 txy2*s2 _ ct  (order matches packing below)
    fields = np.stack([wt, ht, at, tw, th, s2, txy2 * s2, valid, ct],
                      axis=-1)                             # [B,T,9]

    vmask = valid.astype(bool)
    nv_core = [int(vmask[BL * c:BL * (c + 1)].sum()) for c in range(NCORES)]
    blk = max(1, -(-max(nv_core) // 128))

    # batch-63 truths with zero area can't set the noobj mask -> prune
    t63 = truth[B - 1]
    live = (t63[:, 2] * t63[:, 3]) > 0.0
    tn = max(1, int(live.sum()))
    t63l = t63[live][:tn]
    bl_ = t63l[:, 0] - f32(0.5) * t63l[:, 2]
    bu_ = t63l[:, 1] - f32(0.5) * t63l[:, 3]
    br_ = t63l[:, 0] + f32(0.5) * t63l[:, 2]
    bd_ = t63l[:, 1] + f32(0.5) * t63l[:, 3]
    hat3 = (t63l[:, 2] * t63l[:, 3]) / f32(3.0)
    tbrow = np.concatenate([bl_, bu_, br_, bd_, hat3]).astype(ml_dtypes.bfloat16)

    xp63 = x[B - 1].reshape(NUM * AL, HW)

    def fold(vec, n):
        v = np.zeros(blk * 128, f32)
        v[:n] = vec
        return v.reshape(blk, 128).T                       # [128, blk]

    in_maps = []
    pads = []
    b63_pos = None
    for c in range(NCORES):
        bs = slice(BL * c, BL * (c + 1))
        cells = slice(CELLS * c, CELLS * (c + 1))
        m = vmask[bs]                                      # [BL, T]
        n = int(m.sum())

        rw = rows[bs][m]                                   # [n, 16]
        pad = np.broadcast_to(rw[0:1] if n else np.zeros((1, 16), f32),
                              (blk * 128 - n, 16))
        rw512 = np.concatenate([rw, np.ascontiguousarray(pad)])
        rw512[n:, 10:16] = 0.0         # pad obj/vct -> exact constants
        xgc = np.ascontiguousarray(
            rw512.reshape(blk, 128, 16).transpose(1, 0, 2).reshape(128, -1))
        cw = crows[bs][m]                                  # [n, 100] bf16
        cpad = np.zeros((blk * 128 - n, 100), ml_dtypes.bfloat16)
        cw512 = np.concatenate([cw, cpad])
        clsc = np.ascontiguousarray(
            cw512.reshape(blk, 128, 100).transpose(1, 0, 2).reshape(128, -1))

        fc = fields[bs][m]                                 # [n, 9]
        tvw = 7 * blk
        tv = np.zeros((128, tvw), f32)
        tv[:, 0:2 * blk:2] = fold(fc[:, 0], n)             # wt (interleaved)
        tv[:, 1:2 * blk:2] = fold(fc[:, 1], n)             # ht
        tv[:, 2 * blk:3 * blk] = fold(fc[:, 2], n)         # at
        tv[:, 3 * blk:5 * blk:2] = fold(fc[:, 3], n)       # tw
        tv[:, 3 * blk + 1:5 * blk:2] = fold(fc[:, 4], n)   # th
        tv[:, 5 * blk:6 * blk] = fold(fc[:, 5], n)         # s2 (packed->vld=1)
        tv[:, 6 * blk:7 * blk] = fold(fc[:, 6], n)         # txy2*s2

        p63 = np.empty((128, 21), f32)
        for an in range(NUM):
            for cc in range(4):
                p63[:, an * 4 + cc] = xp63[AL * an + cc, cells]
        p63[:, 20] = xp63[4, cells]

        pads.append(blk * 128 - n)
        if c == NCORES - 1:
            start = int(m[:BL - 1].sum())
            b63_pos = (start, int(m[BL - 1].sum()))

        in_maps.append({
            "pf32": np.ascontiguousarray(
                np.concatenate([p63, tv, xgc], axis=1)),
            "pbf": np.ascontiguousarray(np.concatenate(
                [np.broadcast_to(tbrow, (128, 5 * tn)), clsc], axis=1)),
        })
    return in_maps, blk, tn, (b63_pos, pads)


def _combine(results, blk, b63_info):
    b63_pos, pads = b63_info
    npad = float(sum(pads))
    obj = sum(float(r["out"][:, 4:4 + blk].sum(dtype=np.float64))
              for r in results) - npad
    cls = sum(float(r["out"][:, 8:8 + blk].sum(dtype=np.float64))
              for r in results) + (NCORES * blk * 128 - npad)
    noobj = sum(float(r["out"][:, 12].sum(dtype=np.float64)) for r in results)
    sq = sum(float(r["out"][:, 13].sum(dtype=np.float64)) for r in results)
    xy = sum(float(r["out"][:, 14].sum(dtype=np.float64)) for r in results)
    r7 = results[NCORES - 1]["out"]
    start, cnt = b63_pos
    coord63 = 0.0
    for idx in range(start, start + cnt):
        coord63 += float(r7[idx % 128, idx // 128])
    warm = 0.01 * (sq - xy + 0.5 * NUM * HW)
    return np.float32(obj + noobj + warm + coord63 + cls)


def kernel(x, truth, anchors, **_):
    in_maps, blk, tn, b63_pos = _prep(x, truth, anchors)
    key = (blk, tn)
    if key not in _CACHED:
        _CACHED[key] = _build_program(blk, tn)
    nc = _CACHED[key]
    res = run_bass_kernel_spmd(nc, in_maps, list(range(NCORES)))
    return _combine(res.results, blk, b63_pos)
